# revision 13
# baseline (speedup 1.0000x reference)
"""Trainium2 Bass kernel for GQA MultiHeadAttention with RoPE.

Shapes (hardcoded): x (2,2048,1024), Wq (1024,1024), Wk/Wv (1024,256),
Wo (1024,1024). 16 q-heads, 4 kv-heads, head_dim 64.

Sharding: 8 cores = batch (2) x kv-group (4). Core i handles b=i//4,
g=i%4, q-heads {g, 4+g, 8+g, 12+g} (jnp.tile GQA mapping), kv-head g.
Each core emits a partial Y^T (1024,2048) in bf16; the host sums the 4
group partials per batch (f32) and transposes.

Faithful to the reference's multiplicative tril mask before softmax:
  P = exp(mask * (Q K^T) * D**-0.5)   (masked entries = exp(0) = 1)
  out = (P @ V_aug) / Z,  Z carried in V_aug's ones column; fully-masked
  future tiles enter analytically via suffix sums of V fused into the
  normalize pass.

Inner-loop structure (per 128-wide k-tile, all 4 heads at once):
  1 LDW of K-tile shared by 4 S matmuls (free=512 each, two [128,1024]
  f32 PSUM tiles), 2 exp ACTIVATEs of 1024 cols, post-exp mask fix on
  the P tiles (off the scalar critical path), 1 LDW of V-tile shared by
  4 PV matmuls accumulating into 4 [65,512] PSUM accumulators.
"""

import os
import numpy as np
import ml_dtypes

import concourse.bass as bass
import concourse.mybir as mybir
import concourse.tile as tile
from concourse.masks import make_identity
from concourse.bass_utils import run_bass_kernel_spmd

F32 = mybir.dt.float32
DTMM = mybir.dt.bfloat16          # matmul operand dtype
NPMM = ml_dtypes.bfloat16
EXP = mybir.ActivationFunctionType.Exp

B, T, C = 2, 2048, 1024
NH, NKV, D = 16, 4, 64
HG = NH // NKV            # 4 q-heads per kv-group
NQ = 512                  # q chunk width
NCH = T // NQ             # 4 chunks
NKT = T // 128            # 16 k tiles
SCALE = D ** -0.5
PVLAG = 3                 # PV trails S/exp by this many k-tiles


def _split_waits(nc, max_waits=1):
    """This walrus build accepts only one immediate sem-wait per
    instruction; move extras onto preceding same-engine NoOps."""
    for f in nc.m.functions:
        for blk in f.blocks:
            new_insts = []
            for ins in blk.instructions:
                si = ins.sync_info
                if si is not None and len(si.on_wait) > max_waits:
                    waits = list(si.on_wait)
                    extra, keep = waits[:-max_waits], waits[-max_waits:]
                    k = 0
                    while extra:
                        chunk, extra = extra[:max_waits], extra[max_waits:]
                        nop = mybir.InstNoOp(name=f"{ins.name}-ws{k}", ins=[], outs=[])
                        nop.engine = ins.engine
                        nop.sync_info = mybir.SyncInfo(on_wait=chunk, on_update=[])
                        new_insts.append(nop)
                        k += 1
                    si.on_wait = keep
                new_insts.append(ins)
            blk.instructions[:] = new_insts


def _half_swap(eng, dst, src, base):
    """dst rows [base:base+32],[base+32:base+64] = src swapped halves."""
    eng.dma_start(out=dst[base:base + 32, :], in_=src[base + 32:base + 64, :])
    eng.dma_start(out=dst[base + 32:base + 64, :], in_=src[base:base + 32, :])


def _emit(nc, tc, ctx, xT, wq, wkv, wo, ctab, stab, nmsk, yT):
    # ---------- whole-kernel SBUF ----------
    poolW = ctx.enter_context(tc.tile_pool(name="poolW", bufs=1))
    qrotA = poolW.tile([64, NCH * 2 * NQ], DTMM)   # heads 0,1: c*1024 + h*512 + q
    qrotB = poolW.tile([64, NCH * 2 * NQ], DTMM)   # heads 2,3
    kvs = poolW.tile([128, T], DTMM)               # rows 0:64 K-rot, 64:128 V^T
    vaug = poolW.tile([128, NKT * 65], DTMM)
    ostk = [poolW.tile([128, T], DTMM, tag=f"ostk{p}", name=f"ostk{p}")
            for p in range(2)]                     # rows 0:64 head 2p, 64:128 head 2p+1
    sfcol = poolW.tile([64, 4], F32)               # suffix sums of V (col 3 = 0)
    nmskt = poolW.tile([128, 128], mybir.dt.uint8)  # diag triangle (1 = masked)
    onesb = poolW.tile([128, 2 * NQ], DTMM)        # all-ones bf16
    idf = poolW.tile([128, 64], F32)
    idr = poolW.tile([128, 64], DTMM)              # identity both halves, bf16

    nc.gpsimd.memset(idf[:], 0.0)
    make_identity(nc, idf[0:64, :], nomemset=True)
    nc.sync.dma_start(out=idf[64:128, :], in_=idf[0:64, :])
    with nc.allow_low_precision(reason="bf16 constants"):
        nc.vector.tensor_copy(idr[:], idf[:])
    nc.vector.memset(onesb[:], 1.0)

    poolA = ctx.enter_context(tc.tile_pool(name="poolA", bufs=1))
    stg = ctx.enter_context(tc.tile_pool(name="stg", bufs=3))
    pqp = ctx.enter_context(tc.tile_pool(name="pqp", bufs=4))
    nrm = ctx.enter_context(tc.tile_pool(name="nrm", bufs=4))
    dramB = ctx.enter_context(tc.tile_pool(name="dramB", bufs=1, space="DRAM"))
    ps = ctx.enter_context(tc.tile_pool(name="ps", bufs=1, space="PSUM"))

    xtr = [poolA.tile([128, T], DTMM, tag=f"xtr{i}", name=f"xtr{i}")
           for i in range(8)]
    wqr = poolA.tile([128, 8 * 256], DTMM)
    wkvr = poolA.tile([128, 8 * 128], DTMM)
    cost = poolA.tile([128, T], DTMM)
    sint = poolA.tile([128, T], DTMM)
    wor = [poolA.tile([128, C], DTMM, tag=f"wor{p}", name=f"wor{p}")
           for p in range(2)]

    # Bulk loads ride the two hardware DGE queues (sync + scalar); gpsimd's
    # software queue is reserved for small SBUF-SBUF moves. First chunk's x
    # and the weights it needs come first.
    nc.scalar.dma_start(out=wkvr[:], in_=wkv[:])
    nc.sync.dma_start(out=nmskt[:], in_=nmsk[:])
    for i in range(8):
        (nc.sync if i % 2 else nc.scalar).dma_start(
            out=xtr[i][:, 0:NQ], in_=xT[i * 128:(i + 1) * 128, 0:NQ])
    nc.sync.dma_start(out=cost[:], in_=ctab[:])
    nc.scalar.dma_start(out=sint[:], in_=stab[:])
    nc.sync.dma_start(out=wqr[:], in_=wq[:])
    for i in range(8):
        (nc.sync if i % 2 else nc.scalar).dma_start(
            out=xtr[i][:, NQ:2 * NQ], in_=xT[i * 128:(i + 1) * 128, NQ:2 * NQ])
    for i in range(8):
        (nc.sync if i % 2 else nc.scalar).dma_start(
            out=xtr[i][:, 2 * NQ:T], in_=xT[i * 128:(i + 1) * 128, 2 * NQ:T])
    for p in range(2):
        nc.scalar.dma_start(out=wor[p][:], in_=wo[p * 128:(p + 1) * 128, :])

    # HAM warm-up: keep the PE busy on junk matmuls while the first x
    # chunk streams in, so the real matmul stream starts at full clock.
    wps = ps.tile([128, 64], F32, tag="acc", bufs=4, name="wps")
    for w in range(40):
        nc.tensor.matmul(wps[0:64, :], idr[0:64, :], idr[0:64, :],
                         start=True, stop=True)

    # ---------- projections (per chunk, x streams in) ----------
    COPYF = mybir.ActivationFunctionType.Copy

    def emit_kv(c):
        sl = slice(c * NQ, (c + 1) * NQ)
        kvps = ps.tile([128, NQ], F32, tag="acc", bufs=4, name="kvps")
        for i in range(8):
            nc.tensor.matmul(kvps[:], wkvr[:, i * 128:(i + 1) * 128],
                             xtr[i][:, sl], start=(i == 0), stop=(i == 7))
        kcp = stg.tile([128, NQ], DTMM, tag="pcp", name="kcp")
        with nc.allow_low_precision(reason="bf16 K/V"):
            nc.scalar.activation(kcp[:], kvps[:], COPYF)   # K rows + V rows
        nc.vector.tensor_copy(kvs[64:128, sl], kcp[64:128, :])
        swp = stg.tile([128, NQ], DTMM, tag="swp", name="swp")
        _half_swap(nc.sync, swp, kcp, 0)
        t1 = stg.tile([128, NQ], DTMM, tag="t1", name="t1")
        t2 = stg.tile([128, NQ], DTMM, tag="t2", name="t2")
        nc.vector.tensor_mul(t1[0:64, :], kcp[0:64, :], cost[0:64, sl])
        nc.vector.tensor_mul(t2[0:64, :], swp[0:64, :], sint[0:64, sl])
        nc.vector.tensor_add(kvs[0:64, sl], t1[0:64, :], t2[0:64, :])

    def emit_vt(c):
        for kt in range(4 * c, 4 * (c + 1)):
            vtp = ps.tile([128, 64], DTMM, tag="acc", bufs=4, name="vtp")
            with nc.allow_low_precision(reason="bf16 PE transpose of V"):
                nc.tensor.transpose(vtp[:], kvs[64:128, kt * 128:(kt + 1) * 128],
                                    idr[64:128, :])
                nc.vector.tensor_copy(vaug[:, kt * 65:kt * 65 + 64], vtp[:])
            nc.vector.memset(vaug[:, kt * 65 + 64:kt * 65 + 65], 1.0)

    def emit_q(c, p):
        sl = slice(c * NQ, (c + 1) * NQ)
        qdst = qrotA if p == 0 else qrotB
        qps = ps.tile([128, NQ], F32, tag="acc", bufs=4, name="qps")
        for i in range(8):
            nc.tensor.matmul(qps[:], wqr[:, i * 256 + p * 128: i * 256 + (p + 1) * 128],
                             xtr[i][:, sl], start=(i == 0), stop=(i == 7))
        qcp = stg.tile([128, NQ], DTMM, tag="pcp", name="qcp")
        with nc.allow_low_precision(reason="bf16 Q"):
            nc.scalar.activation(qcp[:], qps[:], COPYF)
        swp = stg.tile([128, NQ], DTMM, tag="swp", name="swp")
        _half_swap(nc.sync, swp, qcp, 0)
        _half_swap(nc.scalar, swp, qcp, 64)
        t1 = stg.tile([128, NQ], DTMM, tag="t1", name="t1")
        t2 = stg.tile([128, NQ], DTMM, tag="t2", name="t2")
        nc.vector.tensor_mul(t1[:], qcp[:], cost[:, sl])
        nc.vector.tensor_mul(t2[:], swp[:], sint[:, sl])
        # even head of the pair straight into qrot rows 0:64; odd head via
        # staging + partition-shift DMA
        nc.vector.tensor_add(qdst[0:64, c * 2 * NQ: c * 2 * NQ + NQ],
                             t1[0:64, :], t2[0:64, :])
        qst = stg.tile([128, NQ], DTMM, tag="qst", name="qst")
        nc.vector.tensor_add(qst[64:128, :], t1[64:128, :], t2[64:128, :])
        nc.scalar.dma_start(out=qdst[0:64, c * 2 * NQ + NQ: (c + 1) * 2 * NQ],
                            in_=qst[64:128, :])

    for c in range(NCH):
        emit_kv(c)
        emit_q(c, 0)
        emit_q(c, 1)
        emit_vt(c)

    # suffix sums of V^T along t (for the analytic future-tile term)
    redc = poolA.tile([128, 4], F32)
    nc.gpsimd.memset(redc[:], 0.0)
    for c in range(NCH - 1):
        nc.vector.tensor_reduce(redc[64:128, c:c + 1],
                                kvs[64:128, (c + 1) * NQ:T],
                                axis=mybir.AxisListType.X,
                                op=mybir.AluOpType.add)
    nc.gpsimd.dma_start(out=sfcol[:], in_=redc[64:128, :])

    # ---------- out-projection emitter (interleaved at chunk boundaries;
    # PSUM acc bufs are free between chunks). scalar=True may use the
    # Scalar engine for f32->bf16 copies (only safe after all exp ACTs).
    COPYF2 = mybir.ActivationFunctionType.Copy

    def emit_y(c, use_scalar):
        csl = slice(c * NQ, (c + 1) * NQ)
        for jj in range(2):
            yps = {}
            for p in range(2):
                lhsw = wor[p]
                for j in range(4 * jj, 4 * jj + 4):
                    if j not in yps:
                        yps[j] = ps.tile([128, NQ], F32, tag="acc", bufs=4,
                                         name=f"yps{j}")
                    nc.tensor.matmul(yps[j][:], lhsw[:, j * 128:(j + 1) * 128],
                                     ostk[p][:, csl], start=(p == 0),
                                     stop=(p == 1))
            for j in range(4 * jj, 4 * jj + 4):
                ybf = stg.tile([128, NQ], DTMM, tag="ybf", bufs=4, name="ybf")
                with nc.allow_low_precision(reason="bf16 Y partial"):
                    if use_scalar and j % 2 == 0:
                        nc.scalar.activation(ybf[:], yps[j][:], COPYF2)
                    else:
                        nc.vector.tensor_copy(ybf[:], yps[j][:])
                nc.sync.dma_start(out=yT[j * 128:(j + 1) * 128, csl], in_=ybf[:])

    # ---------- attention ----------
    for c in range(NCH):
        if c >= 2:
            emit_y(c - 2, use_scalar=False)
        csl = slice(c * NQ, (c + 1) * NQ)
        ktiles = 4 * (c + 1)
        # band (diagonal) tiles first, interleaved with history tiles, so the
        # DVE mask fixes spread out and get maximal PV lag slack.
        border = list(range(4 * c, 4 * (c + 1)))
        horder = list(range(0, 4 * c))
        korder = []
        while border or horder:
            if border:
                korder.append(border.pop(0))
            if horder:
                korder.append(horder.pop(0))
        opsh = [ps.tile([65, NQ], F32, tag="acc", bufs=4, name=f"ops{h}")
                for h in range(HG)]
        pqs = {}

        def emit_pv(idx):
            kt = korder[idx]
            pq2 = pqs.pop(kt)
            vAP = vaug[:, kt * 65:(kt + 1) * 65]
            for h in range(HG):
                nc.tensor.matmul(opsh[h][:], vAP,
                                 pq2[:, h * NQ:(h + 1) * NQ],
                                 start=(idx == 0), stop=(idx == ktiles - 1))

        for idx, kt in enumerate(korder):
            kAP = kvs[0:64, kt * 128:(kt + 1) * 128]
            sq01 = ps.tile([128, 2 * NQ], F32, tag="sq", bufs=2, name="sq01")
            sq23 = ps.tile([128, 2 * NQ], F32, tag="sq", bufs=2, name="sq23")
            base = c * 2 * NQ
            nc.tensor.matmul(sq01[:, 0:NQ], kAP,
                             qrotA[0:64, base:base + NQ], start=True, stop=True)
            nc.tensor.matmul(sq01[:, NQ:2 * NQ], kAP,
                             qrotA[0:64, base + NQ:base + 2 * NQ],
                             start=True, stop=True)
            nc.tensor.matmul(sq23[:, 0:NQ], kAP,
                             qrotB[0:64, base:base + NQ], start=True, stop=True)
            nc.tensor.matmul(sq23[:, NQ:2 * NQ], kAP,
                             qrotB[0:64, base + NQ:base + 2 * NQ],
                             start=True, stop=True)
            pq2 = pqp.tile([128, 4 * NQ], DTMM, tag="pq", name="pq")
            nc.scalar.activation(pq2[:, 0:2 * NQ], sq01[:], EXP, scale=SCALE)
            nc.scalar.activation(pq2[:, 2 * NQ:4 * NQ], sq23[:], EXP, scale=SCALE)
            if kt >= 4 * c:       # band tile: masked entries -> exp(0) = 1
                i = kt - 4 * c
                for hs in range(4):   # head slot within pq2
                    off = hs * NQ
                    if i:
                        nc.gpsimd.memset(pq2[:, off:off + i * 128], 1.0)
                    nc.vector.copy_predicated(
                        pq2[:, off + i * 128:off + (i + 1) * 128],
                        nmskt[:], onesb[:, 0:128])
            pqs[kt] = pq2
            if idx >= PVLAG:
                emit_pv(idx - PVLAG)
        for idx in range(max(0, ktiles - PVLAG), ktiles):
            emit_pv(idx)

        # ----- normalize: O = (P@V + suffixV) / (Z + cnt) -----
        cnt = float(T - (c + 1) * NQ)
        for h in range(HG):
            p, odd = h // 2, h % 2
            ocp = nrm.tile([65, NQ], F32, tag="ocp", name="ocp")
            nc.vector.tensor_copy(ocp[:], opsh[h][:])
            zsp = nrm.tile([128, 12], F32, tag="zsp", name="zsp")
            nc.sync.dma_start(
                out=zsp[:, 0:4],
                in_=ocp[64:65, :].rearrange("p (a b) -> p a b", b=4))
            nc.vector.tensor_scalar_add(zsp[:, 4:8], zsp[:, 0:4], cnt)
            nc.vector.reciprocal(zsp[:, 8:12], zsp[:, 4:8])
            zdr = dramB.tile([1, NQ], F32, tag="zdr", bufs=4, name="zdr")
            nc.sync.dma_start(
                out=zdr[:].rearrange("p (a b) -> p a b", b=4),
                in_=zsp[:, 8:12])
            rzb = nrm.tile([64, NQ], F32, tag="rzb", name="rzb")
            nc.sync.dma_start(
                out=rzb[:],
                in_=bass.AP(tensor=zdr.tensor, offset=zdr.offset,
                            ap=[[0, 64]] + [zdr.ap[-1]]))
            with nc.allow_low_precision(reason="bf16 normalized O"):
                if not odd:
                    nc.vector.scalar_tensor_tensor(
                        ostk[p][0:64, csl], ocp[0:64, :], sfcol[:, c:c + 1],
                        rzb[:], op0=mybir.AluOpType.add,
                        op1=mybir.AluOpType.mult)
                else:
                    otm = nrm.tile([64, NQ], DTMM, tag="otm", name="otm")
                    nc.vector.scalar_tensor_tensor(
                        otm[:], ocp[0:64, :], sfcol[:, c:c + 1],
                        rzb[:], op0=mybir.AluOpType.add,
                        op1=mybir.AluOpType.mult)
                    nc.sync.dma_start(out=ostk[p][64:128, csl], in_=otm[:])


    emit_y(NCH - 2, use_scalar=True)
    emit_y(NCH - 1, use_scalar=True)


def _build(nrep=1):
    from contextlib import ExitStack
    nc = bass.Bass()
    xT = nc.declare_dram_parameter("xT", [C, T], DTMM, isOutput=False)
    wq = nc.declare_dram_parameter("wq", [128, 8 * 256], DTMM, isOutput=False)
    wkv = nc.declare_dram_parameter("wkv", [128, 8 * 128], DTMM, isOutput=False)
    wo = nc.declare_dram_parameter("wo", [HG * D, C], DTMM, isOutput=False)
    ctab = nc.declare_dram_parameter("ctab", [128, T], DTMM, isOutput=False)
    stab = nc.declare_dram_parameter("stab", [128, T], DTMM, isOutput=False)
    nmsk = nc.declare_dram_parameter("nmsk", [128, 128], mybir.dt.uint8,
                                     isOutput=False)
    yT = nc.declare_dram_parameter("yT", [C, T], DTMM, isOutput=True)

    with tile.TileContext(nc) as tc:
        for _ in range(nrep):
            with ExitStack() as ctx:
                _emit(nc, tc, ctx, xT, wq, wkv, wo, ctab, stab, nmsk, yT)
    _split_waits(nc)
    return nc


def _host_inputs(x, Wq, Wk, Wv, Wo):
    perm = np.concatenate([np.arange(0, D, 2), np.arange(1, D, 2)])  # even-first
    inv_freq = 1.0 / (10000.0 ** (np.arange(0, D, 2, dtype=np.float64) / D))
    ang = np.arange(T, dtype=np.float64)[:, None] * inv_freq[None, :]
    cos = np.cos(ang).astype(np.float32).T      # (32, T)
    sin = np.sin(ang).astype(np.float32).T
    ctab = np.ascontiguousarray(np.tile(cos, (4, 1)).astype(NPMM))    # (128, T)
    stab = np.ascontiguousarray(
        np.concatenate([-sin, sin, -sin, sin], 0).astype(NPMM))
    # diagonal-block triangle predicate: 1 where k-row is past the q-col
    row = np.arange(128)[:, None]
    q = np.arange(128)[None, :]
    nmsk = np.ascontiguousarray((row > q).astype(np.uint8))

    xTb = [np.ascontiguousarray(x[b].T.astype(NPMM)) for b in range(B)]
    maps = []
    for core in range(8):
        b, g = core // 4, core % 4
        heads = [g + NKV * k for k in range(HG)]
        wq_cols = np.concatenate([h * D + perm for h in heads])
        wq_g = Wq[:, wq_cols].astype(NPMM)
        wq_g = np.ascontiguousarray(
            wq_g.reshape(8, 128, 256).transpose(1, 0, 2).reshape(128, 8 * 256))
        wkv_g = np.concatenate(
            [Wk[:, g * D + perm], Wv[:, g * D:(g + 1) * D]], axis=1).astype(NPMM)
        wkv_g = np.ascontiguousarray(
            wkv_g.reshape(8, 128, 128).transpose(1, 0, 2).reshape(128, 8 * 128))
        wo_rows = np.concatenate([np.arange(h * D, (h + 1) * D) for h in heads])
        wo_g = np.ascontiguousarray(Wo[wo_rows, :].astype(NPMM))
        maps.append({"xT": xTb[b], "wq": wq_g, "wkv": wkv_g, "wo": wo_g,
                     "ctab": ctab, "stab": stab, "nmsk": nmsk})
    return maps


_CACHE = {}


def kernel(x, Wq, Wk, Wv, Wo):
    if "nc" not in _CACHE:
        _CACHE["nc"] = _build()
    nc = _CACHE["nc"]
    maps = _host_inputs(np.asarray(x, np.float32), np.asarray(Wq, np.float32),
                        np.asarray(Wk, np.float32), np.asarray(Wv, np.float32),
                        np.asarray(Wo, np.float32))
    trace = bool(int(os.environ.get("BASSKERNEL_TRACE", "0")))
    res = run_bass_kernel_spmd(nc, maps, list(range(8)), trace=trace)
    if trace and res.exec_time_ns is not None:
        print(f"HW exec time: {res.exec_time_ns} ns")
    out = np.zeros((B, T, C), dtype=np.float32)
    for core in range(8):
        out[core // 4] += res.results[core]["yT"].T.astype(np.float32)
    return out


# revision 14
# speedup vs baseline: 1.1640x; 1.1640x over previous
"""Trainium2 Bass kernel for GQA MultiHeadAttention with RoPE.

Shapes (hardcoded): x (2,2048,1024), Wq (1024,1024), Wk/Wv (1024,256),
Wo (1024,1024). 16 q-heads, 4 kv-heads, head_dim 64.

Sharding: 8 cores = batch (2) x kv-group (4). Core i handles b=i//4,
g=i%4, q-heads {g, 4+g, 8+g, 12+g} (jnp.tile GQA mapping), kv-head g.
Each core emits a partial Y^T (1024,2048) in bf16; the host sums the 4
group partials per batch (f32) and transposes.

Faithful to the reference's multiplicative tril mask before softmax:
  P = exp(mask * (Q K^T) * D**-0.5)   (masked entries = exp(0) = 1)
  out = (P @ V_aug) / Z,  Z carried in V_aug's ones column; fully-masked
  future tiles enter analytically via suffix sums of V fused into the
  normalize pass.

Inner-loop structure (per 128-wide k-tile, all 4 heads at once):
  1 LDW of K-tile shared by 4 S matmuls (free=512 each, two [128,1024]
  f32 PSUM tiles), 2 exp ACTIVATEs of 1024 cols, post-exp mask fix on
  the P tiles (off the scalar critical path), 1 LDW of V-tile shared by
  4 PV matmuls accumulating into 4 [65,512] PSUM accumulators.
"""

import os
import numpy as np
import ml_dtypes

import concourse.bass as bass
import concourse.mybir as mybir
import concourse.tile as tile
from concourse.masks import make_identity
from concourse.bass_utils import run_bass_kernel_spmd

F32 = mybir.dt.float32
DTMM = mybir.dt.bfloat16          # matmul operand dtype
NPMM = ml_dtypes.bfloat16
EXP = mybir.ActivationFunctionType.Exp

B, T, C = 2, 2048, 1024
NH, NKV, D = 16, 4, 64
HG = NH // NKV            # 4 q-heads per kv-group
NQ = 512                  # q chunk width
NCH = T // NQ             # 4 chunks
NKT = T // 128            # 16 k tiles
SCALE = D ** -0.5
PVLAG = 3                 # PV trails S/exp by this many k-tiles


def _split_waits(nc, max_waits=1):
    """This walrus build accepts only one immediate sem-wait per
    instruction; move extras onto preceding same-engine NoOps."""
    for f in nc.m.functions:
        for blk in f.blocks:
            new_insts = []
            for ins in blk.instructions:
                si = ins.sync_info
                if si is not None and len(si.on_wait) > max_waits:
                    waits = list(si.on_wait)
                    extra, keep = waits[:-max_waits], waits[-max_waits:]
                    k = 0
                    while extra:
                        chunk, extra = extra[:max_waits], extra[max_waits:]
                        nop = mybir.InstNoOp(name=f"{ins.name}-ws{k}", ins=[], outs=[])
                        nop.engine = ins.engine
                        nop.sync_info = mybir.SyncInfo(on_wait=chunk, on_update=[])
                        new_insts.append(nop)
                        k += 1
                    si.on_wait = keep
                new_insts.append(ins)
            blk.instructions[:] = new_insts


def _half_swap(eng, dst, src, base):
    """dst rows [base:base+32],[base+32:base+64] = src swapped halves."""
    eng.dma_start(out=dst[base:base + 32, :], in_=src[base + 32:base + 64, :])
    eng.dma_start(out=dst[base + 32:base + 64, :], in_=src[base:base + 32, :])


def _emit(nc, tc, ctx, xT, wq, wkv, wo, ctab, stab, nmsk, yT):
    # ---------- whole-kernel SBUF ----------
    poolW = ctx.enter_context(tc.tile_pool(name="poolW", bufs=1))
    qrotA = poolW.tile([64, NCH * 2 * NQ], DTMM)   # heads 0,1: c*1024 + h*512 + q
    qrotB = poolW.tile([64, NCH * 2 * NQ], DTMM)   # heads 2,3
    kvs = poolW.tile([128, T], DTMM)               # rows 0:64 K-rot, 64:128 V^T
    vaug = poolW.tile([128, NKT * 65], DTMM)
    ostk = [poolW.tile([128, T], DTMM, tag=f"ostk{p}", name=f"ostk{p}")
            for p in range(2)]                     # rows 0:64 head 2p, 64:128 head 2p+1
    sfcol = poolW.tile([64, 4], F32)               # suffix sums of V (col 3 = 0)
    nmskt = poolW.tile([128, 128], mybir.dt.uint8)  # diag triangle (1 = masked)
    onesb = poolW.tile([128, 2 * NQ], DTMM)        # all-ones bf16
    idf = poolW.tile([128, 64], F32)
    idr = poolW.tile([128, 64], DTMM)              # identity both halves, bf16

    nc.gpsimd.memset(idf[:], 0.0)
    make_identity(nc, idf[0:64, :], nomemset=True)
    nc.sync.dma_start(out=idf[64:128, :], in_=idf[0:64, :])
    with nc.allow_low_precision(reason="bf16 constants"):
        nc.vector.tensor_copy(idr[:], idf[:])
    nc.vector.memset(onesb[:], 1.0)

    poolA = ctx.enter_context(tc.tile_pool(name="poolA", bufs=1))
    stg = ctx.enter_context(tc.tile_pool(name="stg", bufs=3))
    pqp = ctx.enter_context(tc.tile_pool(name="pqp", bufs=4))
    nrm = ctx.enter_context(tc.tile_pool(name="nrm", bufs=4))
    dramB = ctx.enter_context(tc.tile_pool(name="dramB", bufs=1, space="DRAM"))
    ps = ctx.enter_context(tc.tile_pool(name="ps", bufs=1, space="PSUM"))

    xtr = [poolA.tile([128, T], DTMM, tag=f"xtr{i}", name=f"xtr{i}")
           for i in range(8)]
    wqr = poolA.tile([128, 8 * 256], DTMM)
    wkvr = poolA.tile([128, 8 * 128], DTMM)
    cost = poolA.tile([128, T], DTMM)
    sint = poolA.tile([128, T], DTMM)
    wor = [poolA.tile([128, C], DTMM, tag=f"wor{p}", name=f"wor{p}")
           for p in range(2)]

    # Bulk loads ride the two hardware DGE queues (sync + scalar); gpsimd's
    # software queue is reserved for small SBUF-SBUF moves. First chunk's x
    # and the weights it needs come first.
    nc.scalar.dma_start(out=wkvr[:], in_=wkv[:])
    nc.sync.dma_start(out=nmskt[:], in_=nmsk[:])
    for i in range(8):
        (nc.sync if i % 2 else nc.scalar).dma_start(
            out=xtr[i][:, 0:NQ], in_=xT[i * 128:(i + 1) * 128, 0:NQ])
    nc.sync.dma_start(out=cost[:], in_=ctab[:])
    nc.scalar.dma_start(out=sint[:], in_=stab[:])
    nc.sync.dma_start(out=wqr[:], in_=wq[:])
    for i in range(8):
        (nc.sync if i % 2 else nc.scalar).dma_start(
            out=xtr[i][:, NQ:2 * NQ], in_=xT[i * 128:(i + 1) * 128, NQ:2 * NQ])
    for i in range(8):
        (nc.sync if i % 2 else nc.scalar).dma_start(
            out=xtr[i][:, 2 * NQ:T], in_=xT[i * 128:(i + 1) * 128, 2 * NQ:T])
    for p in range(2):
        nc.scalar.dma_start(out=wor[p][:], in_=wo[p * 128:(p + 1) * 128, :])

    # HAM warm-up: keep the PE busy on junk matmuls while the first x
    # chunk streams in, so the real matmul stream starts at full clock.
    wps = ps.tile([128, 64], F32, tag="acc", bufs=4, name="wps")
    for w in range(40):
        nc.tensor.matmul(wps[0:64, :], idr[0:64, :], idr[0:64, :],
                         start=True, stop=True)

    # ---------- projections (per chunk, x streams in) ----------
    COPYF = mybir.ActivationFunctionType.Copy

    def emit_kv(c):
        sl = slice(c * NQ, (c + 1) * NQ)
        kvps = ps.tile([128, NQ], F32, tag="acc", bufs=4, name="kvps")
        for i in range(8):
            nc.tensor.matmul(kvps[:], wkvr[:, i * 128:(i + 1) * 128],
                             xtr[i][:, sl], start=(i == 0), stop=(i == 7))
        kcp = stg.tile([128, NQ], DTMM, tag="pcp", name="kcp")
        with nc.allow_low_precision(reason="bf16 K/V"):
            nc.scalar.activation(kcp[:], kvps[:], COPYF)   # K rows + V rows
        nc.vector.tensor_copy(kvs[64:128, sl], kcp[64:128, :])
        swp = stg.tile([128, NQ], DTMM, tag="swp", name="swp")
        _half_swap(nc.sync, swp, kcp, 0)
        t1 = stg.tile([128, NQ], DTMM, tag="t1", name="t1")
        t2 = stg.tile([128, NQ], DTMM, tag="t2", name="t2")
        nc.vector.tensor_mul(t1[0:64, :], kcp[0:64, :], cost[0:64, sl])
        nc.vector.tensor_mul(t2[0:64, :], swp[0:64, :], sint[0:64, sl])
        nc.vector.tensor_add(kvs[0:64, sl], t1[0:64, :], t2[0:64, :])

    def emit_vt(c):
        for kt in range(4 * c, 4 * (c + 1)):
            vtp = ps.tile([128, 64], DTMM, tag="acc", bufs=4, name="vtp")
            with nc.allow_low_precision(reason="bf16 PE transpose of V"):
                nc.tensor.transpose(vtp[:], kvs[64:128, kt * 128:(kt + 1) * 128],
                                    idr[64:128, :])
                nc.vector.tensor_copy(vaug[:, kt * 65:kt * 65 + 64], vtp[:])
            nc.vector.memset(vaug[:, kt * 65 + 64:kt * 65 + 65], 1.0)

    def emit_q(c, p):
        sl = slice(c * NQ, (c + 1) * NQ)
        qdst = qrotA if p == 0 else qrotB
        qps = ps.tile([128, NQ], F32, tag="acc", bufs=4, name="qps")
        for i in range(8):
            nc.tensor.matmul(qps[:], wqr[:, i * 256 + p * 128: i * 256 + (p + 1) * 128],
                             xtr[i][:, sl], start=(i == 0), stop=(i == 7))
        qcp = stg.tile([128, NQ], DTMM, tag="pcp", name="qcp")
        with nc.allow_low_precision(reason="bf16 Q"):
            nc.scalar.activation(qcp[:], qps[:], COPYF)
        swp = stg.tile([128, NQ], DTMM, tag="swp", name="swp")
        _half_swap(nc.sync, swp, qcp, 0)
        _half_swap(nc.scalar, swp, qcp, 64)
        t1 = stg.tile([128, NQ], DTMM, tag="t1", name="t1")
        t2 = stg.tile([128, NQ], DTMM, tag="t2", name="t2")
        nc.vector.tensor_mul(t1[:], qcp[:], cost[:, sl])
        nc.vector.tensor_mul(t2[:], swp[:], sint[:, sl])
        # even head of the pair straight into qrot rows 0:64; odd head via
        # staging + partition-shift DMA
        nc.vector.tensor_add(qdst[0:64, c * 2 * NQ: c * 2 * NQ + NQ],
                             t1[0:64, :], t2[0:64, :])
        qst = stg.tile([128, NQ], DTMM, tag="qst", name="qst")
        nc.vector.tensor_add(qst[64:128, :], t1[64:128, :], t2[64:128, :])
        nc.scalar.dma_start(out=qdst[0:64, c * 2 * NQ + NQ: (c + 1) * 2 * NQ],
                            in_=qst[64:128, :])

    for c in range(NCH):
        emit_kv(c)
        emit_q(c, 0)
        emit_q(c, 1)
        emit_vt(c)

    # suffix sums of V^T along t (for the analytic future-tile term)
    redc = poolA.tile([128, 4], F32)
    nc.gpsimd.memset(redc[:], 0.0)
    for c in range(NCH - 1):
        nc.vector.tensor_reduce(redc[64:128, c:c + 1],
                                kvs[64:128, (c + 1) * NQ:T],
                                axis=mybir.AxisListType.X,
                                op=mybir.AluOpType.add)
    nc.gpsimd.dma_start(out=sfcol[:], in_=redc[64:128, :])

    # ---------- out-projection emitter (interleaved at chunk boundaries;
    # PSUM acc bufs are free between chunks). scalar=True may use the
    # Scalar engine for f32->bf16 copies (only safe after all exp ACTs).
    COPYF2 = mybir.ActivationFunctionType.Copy

    def emit_y(c, use_scalar):
        csl = slice(c * NQ, (c + 1) * NQ)
        for jj in range(2):
            ya = ps.tile([128, 2 * NQ], F32, tag="sq", bufs=2, name="ya")
            yb = ps.tile([128, 2 * NQ], F32, tag="sq", bufs=2, name="yb")
            halves = {4 * jj + 0: ya[:, 0:NQ], 4 * jj + 1: ya[:, NQ:2 * NQ],
                      4 * jj + 2: yb[:, 0:NQ], 4 * jj + 3: yb[:, NQ:2 * NQ]}
            for p in range(2):
                lhsw = wor[p]
                for j, half in halves.items():
                    nc.tensor.matmul(half, lhsw[:, j * 128:(j + 1) * 128],
                                     ostk[p][:, csl], start=(p == 0),
                                     stop=(p == 1))
            for j, half in halves.items():
                ybf = stg.tile([128, NQ], DTMM, tag="ybf", bufs=4, name="ybf")
                with nc.allow_low_precision(reason="bf16 Y partial"):
                    if use_scalar and j % 2 == 0:
                        nc.scalar.activation(ybf[:], half, COPYF2)
                    else:
                        nc.vector.tensor_copy(ybf[:], half)
                nc.sync.dma_start(out=yT[j * 128:(j + 1) * 128, csl], in_=ybf[:])

    # ---------- attention ----------
    for c in range(NCH):
        csl = slice(c * NQ, (c + 1) * NQ)
        ktiles = 4 * (c + 1)
        # band (diagonal) tiles first, interleaved with history tiles, so the
        # DVE mask fixes spread out and get maximal PV lag slack.
        border = list(range(4 * c, 4 * (c + 1)))
        horder = list(range(0, 4 * c))
        korder = []
        while border or horder:
            if border:
                korder.append(border.pop(0))
            if horder:
                korder.append(horder.pop(0))
        opsh = [ps.tile([65, NQ], F32, tag="acc", bufs=4, name=f"ops{h}")
                for h in range(HG)]
        pqs = {}

        def emit_pv(idx):
            kt = korder[idx]
            pq2 = pqs.pop(kt)
            vAP = vaug[:, kt * 65:(kt + 1) * 65]
            for h in range(HG):
                nc.tensor.matmul(opsh[h][:], vAP,
                                 pq2[:, h * NQ:(h + 1) * NQ],
                                 start=(idx == 0), stop=(idx == ktiles - 1))

        for idx, kt in enumerate(korder):
            kAP = kvs[0:64, kt * 128:(kt + 1) * 128]
            sq01 = ps.tile([128, 2 * NQ], F32, tag="sq", bufs=2, name="sq01")
            sq23 = ps.tile([128, 2 * NQ], F32, tag="sq", bufs=2, name="sq23")
            base = c * 2 * NQ
            nc.tensor.matmul(sq01[:, 0:NQ], kAP,
                             qrotA[0:64, base:base + NQ], start=True, stop=True)
            nc.tensor.matmul(sq01[:, NQ:2 * NQ], kAP,
                             qrotA[0:64, base + NQ:base + 2 * NQ],
                             start=True, stop=True)
            nc.tensor.matmul(sq23[:, 0:NQ], kAP,
                             qrotB[0:64, base:base + NQ], start=True, stop=True)
            nc.tensor.matmul(sq23[:, NQ:2 * NQ], kAP,
                             qrotB[0:64, base + NQ:base + 2 * NQ],
                             start=True, stop=True)
            pq2 = pqp.tile([128, 4 * NQ], DTMM, tag="pq", name="pq")
            nc.scalar.activation(pq2[:, 0:2 * NQ], sq01[:], EXP, scale=SCALE)
            nc.scalar.activation(pq2[:, 2 * NQ:4 * NQ], sq23[:], EXP, scale=SCALE)
            if kt >= 4 * c:       # band tile: masked entries -> exp(0) = 1
                i = kt - 4 * c
                for hs in range(4):   # head slot within pq2
                    off = hs * NQ
                    if i:
                        nc.gpsimd.memset(pq2[:, off:off + i * 128], 1.0)
                    nc.vector.copy_predicated(
                        pq2[:, off + i * 128:off + (i + 1) * 128],
                        nmskt[:], onesb[:, 0:128])
            pqs[kt] = pq2
            if idx >= PVLAG:
                emit_pv(idx - PVLAG)
        for idx in range(max(0, ktiles - PVLAG), ktiles):
            emit_pv(idx)

        # ----- normalize: O = (P@V + suffixV) / (Z + cnt) -----
        cnt = float(T - (c + 1) * NQ)
        for h in range(HG):
            p, odd = h // 2, h % 2
            ocp = nrm.tile([65, NQ], F32, tag="ocp", name="ocp")
            nc.vector.tensor_copy(ocp[:], opsh[h][:])
            zsp = nrm.tile([128, 12], F32, tag="zsp", name="zsp")
            nc.sync.dma_start(
                out=zsp[:, 0:4],
                in_=ocp[64:65, :].rearrange("p (a b) -> p a b", b=4))
            nc.vector.tensor_scalar_add(zsp[:, 4:8], zsp[:, 0:4], cnt)
            nc.vector.reciprocal(zsp[:, 8:12], zsp[:, 4:8])
            zdr = dramB.tile([1, NQ], F32, tag="zdr", bufs=4, name="zdr")
            nc.sync.dma_start(
                out=zdr[:].rearrange("p (a b) -> p a b", b=4),
                in_=zsp[:, 8:12])
            rzb = nrm.tile([64, NQ], F32, tag="rzb", name="rzb")
            nc.sync.dma_start(
                out=rzb[:],
                in_=bass.AP(tensor=zdr.tensor, offset=zdr.offset,
                            ap=[[0, 64]] + [zdr.ap[-1]]))
            with nc.allow_low_precision(reason="bf16 normalized O"):
                if not odd:
                    nc.vector.scalar_tensor_tensor(
                        ostk[p][0:64, csl], ocp[0:64, :], sfcol[:, c:c + 1],
                        rzb[:], op0=mybir.AluOpType.add,
                        op1=mybir.AluOpType.mult)
                else:
                    otm = nrm.tile([64, NQ], DTMM, tag="otm", name="otm")
                    nc.vector.scalar_tensor_tensor(
                        otm[:], ocp[0:64, :], sfcol[:, c:c + 1],
                        rzb[:], op0=mybir.AluOpType.add,
                        op1=mybir.AluOpType.mult)
                    nc.sync.dma_start(out=ostk[p][64:128, csl], in_=otm[:])


    for c in range(NCH):
        emit_y(c, use_scalar=True)


def _build(nrep=1):
    from contextlib import ExitStack
    nc = bass.Bass()
    xT = nc.declare_dram_parameter("xT", [C, T], DTMM, isOutput=False)
    wq = nc.declare_dram_parameter("wq", [128, 8 * 256], DTMM, isOutput=False)
    wkv = nc.declare_dram_parameter("wkv", [128, 8 * 128], DTMM, isOutput=False)
    wo = nc.declare_dram_parameter("wo", [HG * D, C], DTMM, isOutput=False)
    ctab = nc.declare_dram_parameter("ctab", [128, T], DTMM, isOutput=False)
    stab = nc.declare_dram_parameter("stab", [128, T], DTMM, isOutput=False)
    nmsk = nc.declare_dram_parameter("nmsk", [128, 128], mybir.dt.uint8,
                                     isOutput=False)
    yT = nc.declare_dram_parameter("yT", [C, T], DTMM, isOutput=True)

    with tile.TileContext(nc) as tc:
        for _ in range(nrep):
            with ExitStack() as ctx:
                _emit(nc, tc, ctx, xT, wq, wkv, wo, ctab, stab, nmsk, yT)
    _split_waits(nc)
    return nc


def _host_inputs(x, Wq, Wk, Wv, Wo):
    perm = np.concatenate([np.arange(0, D, 2), np.arange(1, D, 2)])  # even-first
    inv_freq = 1.0 / (10000.0 ** (np.arange(0, D, 2, dtype=np.float64) / D))
    ang = np.arange(T, dtype=np.float64)[:, None] * inv_freq[None, :]
    cos = np.cos(ang).astype(np.float32).T      # (32, T)
    sin = np.sin(ang).astype(np.float32).T
    ctab = np.ascontiguousarray(np.tile(cos, (4, 1)).astype(NPMM))    # (128, T)
    stab = np.ascontiguousarray(
        np.concatenate([-sin, sin, -sin, sin], 0).astype(NPMM))
    # diagonal-block triangle predicate: 1 where k-row is past the q-col
    row = np.arange(128)[:, None]
    q = np.arange(128)[None, :]
    nmsk = np.ascontiguousarray((row > q).astype(np.uint8))

    xTb = [np.ascontiguousarray(x[b].T.astype(NPMM)) for b in range(B)]
    maps = []
    for core in range(8):
        b, g = core // 4, core % 4
        heads = [g + NKV * k for k in range(HG)]
        wq_cols = np.concatenate([h * D + perm for h in heads])
        wq_g = Wq[:, wq_cols].astype(NPMM)
        wq_g = np.ascontiguousarray(
            wq_g.reshape(8, 128, 256).transpose(1, 0, 2).reshape(128, 8 * 256))
        wkv_g = np.concatenate(
            [Wk[:, g * D + perm], Wv[:, g * D:(g + 1) * D]], axis=1).astype(NPMM)
        wkv_g = np.ascontiguousarray(
            wkv_g.reshape(8, 128, 128).transpose(1, 0, 2).reshape(128, 8 * 128))
        wo_rows = np.concatenate([np.arange(h * D, (h + 1) * D) for h in heads])
        wo_g = np.ascontiguousarray(Wo[wo_rows, :].astype(NPMM))
        maps.append({"xT": xTb[b], "wq": wq_g, "wkv": wkv_g, "wo": wo_g,
                     "ctab": ctab, "stab": stab, "nmsk": nmsk})
    return maps


_CACHE = {}


def kernel(x, Wq, Wk, Wv, Wo):
    if "nc" not in _CACHE:
        _CACHE["nc"] = _build()
    nc = _CACHE["nc"]
    maps = _host_inputs(np.asarray(x, np.float32), np.asarray(Wq, np.float32),
                        np.asarray(Wk, np.float32), np.asarray(Wv, np.float32),
                        np.asarray(Wo, np.float32))
    trace = bool(int(os.environ.get("BASSKERNEL_TRACE", "0")))
    res = run_bass_kernel_spmd(nc, maps, list(range(8)), trace=trace)
    if trace and res.exec_time_ns is not None:
        print(f"HW exec time: {res.exec_time_ns} ns")
    out = np.zeros((B, T, C), dtype=np.float32)
    for core in range(8):
        out[core // 4] += res.results[core]["yT"].T.astype(np.float32)
    return out
